# revision 50
# baseline (speedup 1.0000x reference)
"""Multi-head causal attention (B=1, T=4096, D=768, H=12) on 8 trn2 cores.

v3 sharding (balanced, no dummy slots): every core runs the identical
SPMD program with two head-slots:
  slot A (QKV partition rows 0:64) = one of heads 0..7, full attention,
    normalized + out-projected on device -> partial outT (bf16).
  slot B (rows 64:128) = one of heads 8..11, KEY-SPLIT in half across a
    core pair: core parity p handles natural key chunks {2j+p}.  The
    host packs xT_B = x restricted to those 2048 tokens, so on device
    slot B sees a dense 2048-key sequence; for query tile qi it needs
    exactly 2(qi+1) key chunks on every core (shape-uniform).  Slot B's
    unnormalized AV + denominator go back raw (outB); the host joins the
    two halves (av0+av1)/(d0+d1) and applies that head's out-projection
    (a tiny [4096,64]@[64,768] BLAS call per split head).

Per-core causal work: 1 + 0.5 heads = 12/8 of a head -- balanced.

Other structure (from v2): batched DVE reciprocals, gpsimd stride-0
broadcast of 1/denom, av evacuated with one [128,1024] copy into an f32
staging buffer, bf16 outT.
"""

import math
import numpy as np
import ml_dtypes
from contextlib import ExitStack

import concourse.bass as bass
import concourse.bacc as bacc
import concourse.mybir as mybir
import concourse.tile as tile
from concourse.bass_utils import run_bass_kernel_spmd

BF16 = mybir.dt.bfloat16
F32 = mybir.dt.float32
AF = mybir.ActivationFunctionType

T = 4096
TB = T // 2               # slot-B dense key space
D_MODEL = 768
HEAD_DIM = 64
N_HEADS = 12
N_CORES = 8
QT = 512                  # query tile width (one full PSUM bank per chunk)
KC = 128                  # key chunk (psum partition dim)
GRP = 3                   # score chunk-jobs per exp group -> ACT free dim 1536
NQT = T // QT             # 8 query tiles
CCH = D_MODEL // 128      # 6 contraction chunks
TOKT = 512                # token tile for projections
NTOKT = T // TOKT
NTOKTB = TB // TOKT
VST = 130                 # V storage stride per 128-token chunk

_PROGRAM_CACHE = {}


def build_program():
    nc = bacc.Bacc(None)

    # weights/masks arrive host-packed in device layout: one DMA each
    # (every dma_start costs ~650ns of ISSUE time on its queue; 33 small
    # startup DMAs serialized to ~21us of PE-idle before the first matmul)
    xT_d = nc.declare_dram_parameter("xT", [D_MODEL, T], BF16, isOutput=False)
    xTB_d = nc.declare_dram_parameter("xTB", [D_MODEL, TB], BF16, isOutput=False)
    w_d = nc.declare_dram_parameter("wqkv", [128, 3 * CCH * 128], BF16, isOutput=False)
    b_d = nc.declare_dram_parameter("bqkv", [128, 3], F32, isOutput=False)
    wo_d = nc.declare_dram_parameter("wo2", [128, D_MODEL], BF16, isOutput=False)
    mk_d = nc.declare_dram_parameter("masks", [128, 4 * QT], BF16, isOutput=False)
    mkB_d = nc.declare_dram_parameter("masksB", [128, 2 * QT], BF16, isOutput=False)
    id_d = nc.declare_dram_parameter("ident", [128, 128], BF16, isOutput=False)
    outT_d = nc.declare_dram_parameter("outT", [D_MODEL, T], BF16, isOutput=True)
    outB_d = nc.declare_dram_parameter("outB", [65, T], F32, isOutput=True)

    with tile.TileContext(nc) as tc, ExitStack() as ctx:
        consts = ctx.enter_context(tc.tile_pool(name="consts", bufs=1))
        big = ctx.enter_context(tc.tile_pool(name="big", bufs=1))
        ptp = ctx.enter_context(tc.tile_pool(name="ptp", bufs=4))
        osb = ctx.enter_context(tc.tile_pool(name="osb", bufs=3))
        rbp = ctx.enter_context(tc.tile_pool(name="rbp", bufs=8))
        # PSUM: scores/proj/outproj pool 3 banks x2, av 2 banks x1 = 8
        sp = ctx.enter_context(tc.tile_pool(name="sp", bufs=2, space="PSUM"))
        avp = ctx.enter_context(tc.tile_pool(name="avp", bufs=1, space="PSUM"))
        dramp = ctx.enter_context(tc.tile_pool(name="dramp", bufs=1, space="DRAM"))

        # ---- constants / inputs to SBUF ----
        # issue order: weights first (first matmul needs them), then xT on
        # sync; xTB via the idle gpsimd queue; late-needed consts via scalar
        # w/b on the scalar queue: queue transfers are FIFO, so on sync
        # they would serialize ahead of xT[0] and delay the first matmul
        w_sb = consts.tile([128, 3 * CCH * 128], BF16, tag="w")
        nc.scalar.dma_start(out=w_sb[:], in_=w_d[:, :])
        b_sb = consts.tile([128, 3], F32, tag="b")
        nc.scalar.dma_start(out=b_sb[:], in_=b_d[:, :])
        # xT in halves: projections for token tiles 0-3 start once the first
        # 3.15MB lands instead of waiting for all of xT; xTB queues behind
        # xT on the same engine so it cannot steal HBM from the critical path
        HT = T // 2
        xT_sb = []
        for j in range(CCH):
            t = big.tile([128, T], BF16, tag=f"xT{j}")
            nc.sync.dma_start(
                out=t[:, 0:HT], in_=xT_d[j * 128:(j + 1) * 128, 0:HT])
            xT_sb.append(t)
        for j in range(CCH):
            nc.sync.dma_start(
                out=xT_sb[j][:, HT:T], in_=xT_d[j * 128:(j + 1) * 128, HT:T])
        xTB_sb = []
        for j in range(CCH):
            t = big.tile([128, TB], BF16, tag=f"xTB{j}")
            nc.sync.dma_start(out=t[:], in_=xTB_d[j * 128:(j + 1) * 128, :])
            xTB_sb.append(t)
        wo_sb = consts.tile([128, D_MODEL], BF16, tag="wo")
        nc.scalar.dma_start(out=wo_sb[:], in_=wo_d[:, :])
        mask_sb = consts.tile([128, 4 * QT], BF16, tag="mask")
        nc.scalar.dma_start(out=mask_sb[:], in_=mk_d[:, :])
        maskB_sb = consts.tile([128, 2 * QT], BF16, tag="maskB")
        nc.scalar.dma_start(out=maskB_sb[:], in_=mkB_d[:, :])
        id_sb = consts.tile([128, 128], BF16, tag="id")
        nc.scalar.dma_start(out=id_sb[:], in_=id_d[:, :])

        # ---- projections ----
        # Q: both slots read xT (slot B attends from all 4096 queries).
        # K/V: slot A from xT (4096 keys), slot B from xTB (2048 dense keys).
        qkv_sb = []
        for s in range(3):
            t = big.tile([128, T], BF16, tag=f"qkv{s}")
            qkv_sb.append(t)
        # slot B's K/V region beyond TB is never projected; zero V there so
        # the shared PE transposes read defined data
        nc.vector.memset(qkv_sb[2][64:128, TB:T], 0.0)

        def mm_split(out, lhsT, rhs, first, last, col=0):
            nc.tensor.matmul(
                out, lhsT, rhs, start=first, stop=last,
                tile_position=(0, col),
            )

        for tt in range(NTOKT):
            pp = sp.tile([128, TOKT], F32, tag="sc")
            for j in range(CCH):
                base = j * 128
                mm_split(pp[:, :], w_sb[:, base:base + 128],
                         xT_sb[j][:, tt * TOKT:(tt + 1) * TOKT],
                         j == 0, j == CCH - 1)
            nc.vector.tensor_scalar_add(
                qkv_sb[0][:, tt * TOKT:(tt + 1) * TOKT],
                pp[:, :], b_sb[:, 0:1],
            )
        for s in (1, 2):
            for tt in range(NTOKT):
                if tt < NTOKTB:
                    pp = sp.tile([128, 2 * TOKT], F32, tag="sc")
                    for j in range(CCH):
                        base = (s * CCH + j) * 128
                        mm_split(pp[0:64, 0:TOKT], w_sb[:, base:base + 64],
                                 xT_sb[j][:, tt * TOKT:(tt + 1) * TOKT],
                                 j == 0, j == CCH - 1)
                        mm_split(pp[64:128, TOKT:2 * TOKT],
                                 w_sb[:, base + 64:base + 128],
                                 xTB_sb[j][:, tt * TOKT:(tt + 1) * TOKT],
                                 j == 0, j == CCH - 1, col=64)
                    nc.vector.tensor_scalar_add(
                        qkv_sb[s][0:64, tt * TOKT:(tt + 1) * TOKT],
                        pp[0:64, 0:TOKT], b_sb[0:64, s:s + 1],
                    )
                    nc.vector.tensor_scalar_add(
                        qkv_sb[s][64:128, tt * TOKT:(tt + 1) * TOKT],
                        pp[64:128, TOKT:2 * TOKT], b_sb[64:128, s:s + 1],
                    )
                else:
                    pp = sp.tile([128, TOKT], F32, tag="sc")
                    for j in range(CCH):
                        base = (s * CCH + j) * 128
                        mm_split(pp[0:64, :], w_sb[:, base:base + 64],
                                 xT_sb[j][:, tt * TOKT:(tt + 1) * TOKT],
                                 j == 0, j == CCH - 1)
                    nc.vector.tensor_scalar_add(
                        qkv_sb[s][0:64, tt * TOKT:(tt + 1) * TOKT],
                        pp[0:64, :], b_sb[0:64, s:s + 1],
                    )
        QT_sb, KT_sb, VT_sb = qkv_sb

        # ---- V2 per 128-token key tile, stride 208 cols:
        #   [0:64]=V_A  [64:65]=1  [65:129]=V_B  [129:130]=1
        # lhsT A = cols 0:65  -> psum rows 0:64 AV_A, row 64 denom_A
        # lhsT B = cols 65:130 -> psum rows 0:64 AV_B, row 64 denom_B
        # (both lhsTs 65-wide: LDWEIGHTS cost is per-column, 54ns not 107)
        V_sb = big.tile([128, (T // 128) * VST], BF16, tag="V")
        nc.vector.memset(V_sb[:], 0.0)
        v3 = V_sb[:].rearrange("p (t c) -> p t c", c=VST)
        nc.vector.memset(v3[:, :, 64:65], 1.0)
        nc.vector.memset(v3[:, :, 129:130], 1.0)
        for tt4 in range(T // 128):
            tp = sp.tile([128, 128], BF16, tag="sc")
            nc.tensor.transpose(tp[:], VT_sb[:, tt4 * 128:(tt4 + 1) * 128], id_sb[:])
            nc.vector.tensor_copy(V_sb[:, tt4 * VST:tt4 * VST + 64], tp[:, 0:64])
            nc.vector.tensor_copy(
                V_sb[:, tt4 * VST + 65:tt4 * VST + 129], tp[:, 64:128])

        # ---- attention: scores -> exp -> mask -> AV, denoms staged ----
        # denominator staging: engine ops need base partition 0/32/64/96,
        # so qtiles 0-3 stage at partitions 0:4 and qtiles 4-7 at 32:36
        avs_all = big.tile([128, NQT * 2 * QT], F32, tag="avs")
        den_sb = big.tile([36, QT], F32, tag="den")
        r_sb = big.tile([36, QT], F32, tag="r")
        ho_all = big.tile([64, T], BF16, tag="ho")
        rd = dramp.tile([36, QT], F32, tag="rd")

        def den_row(qi):
            return (0 if qi < 4 else 32) + (qi % 4)

        for qi in range(NQT):
            qs = qi * QT
            av = avp.tile([128, 2 * QT], F32, tag="av")
            nsteps = 4 * (qi + 1)
            nstepsB = 2 * (qi + 1)
            jobs = []
            for i in range(nsteps):
                jobs.append((i, 0))
                if i < nstepsB:
                    jobs.append((i, 1))
            for g in range(0, len(jobs), GRP):
                grp = jobs[g:g + GRP]
                width = len(grp) * QT
                sc = sp.tile([128, GRP * QT], F32, tag="sc")
                for ji, (kc, h) in enumerate(grp):
                    nc.tensor.matmul(
                        sc[:, ji * QT:(ji + 1) * QT],
                        KT_sb[64 * h:64 * h + 64, kc * KC:(kc + 1) * KC],
                        QT_sb[64 * h:64 * h + 64, qs:qs + QT],
                        start=True, stop=True, tile_position=(64 * h, 0),
                    )
                pt = ptp.tile([128, GRP * QT], BF16, tag="pt")
                nc.scalar.activation(
                    pt[:, :width], sc[:, :width], AF.Exp,
                    scale=1.0 / math.sqrt(HEAD_DIM),
                )
                for ji, (kc, h) in enumerate(grp):
                    ptj = pt[:, ji * QT:(ji + 1) * QT]
                    if h == 0:
                        if kc >= 4 * qi:  # diagonal straddle
                            pat = kc - 4 * qi
                            m = mask_sb[:, pat * QT:(pat + 1) * QT]
                            nc.vector.tensor_mul(ptj, ptj, m)
                        vbase = kc * VST
                        nc.tensor.matmul(
                            av[0:65, 0:QT], V_sb[:, vbase:vbase + 65], ptj,
                            start=(kc == 0), stop=(kc == nsteps - 1),
                            tile_position=(0, 0),
                        )
                    else:
                        if kc >= 2 * qi:  # diagonal straddle (2 per qtile)
                            pat = kc - 2 * qi
                            m = maskB_sb[:, pat * QT:(pat + 1) * QT]
                            nc.vector.tensor_mul(ptj, ptj, m)
                        vbase = kc * VST
                        nc.tensor.matmul(
                            av[0:65, QT:2 * QT], V_sb[:, vbase + 65:vbase + 130],
                            ptj, start=(kc == 0), stop=(kc == nstepsB - 1),
                            tile_position=(0, 0),
                        )
            # evacuate av bank pair with one big copy; stage A denom row
            avs = avs_all[:, qi * 2 * QT:(qi + 1) * 2 * QT]
            nc.vector.tensor_copy(avs, av[:, :])
            # DVE lanes cannot cross partitions; gather denom rows into the
            # base-aligned staging tile with tiny SBUF->SBUF DMAs
            ra = den_row(qi)
            nc.sync.dma_start(out=den_sb[ra:ra + 1, :], in_=avs[64:65, 0:QT])
            # slot B raw output (unnormalized av rows 0:64 + denom row 64)
            nc.sync.dma_start(
                out=outB_d[0:65, qs:qs + QT], in_=avs[0:65, QT:2 * QT])
            if qi == 3:
                nc.vector.reciprocal(r_sb[0:4, :], den_sb[0:4, :])
                nc.sync.dma_start(out=rd[0:4, :], in_=r_sb[0:4, :])
            if qi == 7:
                nc.vector.reciprocal(r_sb[32:36, :], den_sb[32:36, :])
                nc.sync.dma_start(out=rd[32:36, :], in_=r_sb[32:36, :])

        # ---- phase B: normalize + out-projection (slot A only) ----
        for qi in range(NQT):
            qs = qi * QT
            avs = avs_all[:, qi * 2 * QT:(qi + 1) * 2 * QT]
            rbc = rbp.tile([64, QT], F32, tag="rbc")
            ra = den_row(qi)
            rdA = rd[ra:ra + 1, :]
            nc.gpsimd.dma_start(
                out=rbc[0:64, :],
                in_=bass.AP(tensor=rdA.tensor, offset=rdA.offset,
                            ap=[[0, 64]] + list(rdA.ap[1:])))
            nc.vector.tensor_mul(
                ho_all[0:64, qs:qs + QT], avs[0:64, 0:QT], rbc[0:64, :])
            # 3 dout-chunks per PSUM tile -> one cast per tile
            for dg in range(CCH // 3):
                op = sp.tile([128, 3 * QT], F32, tag="sc")
                for i in range(3):
                    dch = dg * 3 + i
                    nc.tensor.matmul(
                        op[:, i * QT:(i + 1) * QT],
                        wo_sb[0:64, dch * 128:(dch + 1) * 128],
                        ho_all[0:64, qs:qs + QT], start=True, stop=True,
                    )
                ot = osb.tile([128, 3 * QT], BF16, tag="ot")
                nc.vector.tensor_copy(ot[:], op[:, :])
                # single store: dst viewed [128p, 3i, 512q] to match src
                nc.sync.dma_start(
                    out=outT_d[dg * 384:(dg + 1) * 384, qs:qs + QT]
                        .rearrange("(i p) q -> p i q", i=3),
                    in_=ot[:].rearrange("p (i q) -> p i q", i=3),
                )
    nc.finalize()
    return nc


def _host_inputs(x, wq, bq, wk, bk, wv, bv, wo):
    """Per-core input maps. Slot A of core c = head c (full); slot B =
    head 8 + c//2, key-parity c&1 (half keys, host-packed dense)."""
    bf16 = ml_dtypes.bfloat16
    x0 = np.asarray(x[0], np.float32)              # [T, D]
    xT = np.ascontiguousarray(x0.T).astype(bf16)
    chunks = x0.reshape(T // 128, 128, D_MODEL)
    masks = np.zeros((4, 128, QT), np.float32)
    dk = np.arange(128)[:, None]
    dq = np.arange(QT)[None, :]
    for p in range(4):
        masks[p] = (dk + 128 * p <= dq)
    masks = masks.astype(bf16)
    # device layout [128, pat*QT+q] so each mask set loads as one DMA
    masks_packed = np.ascontiguousarray(
        masks.transpose(1, 0, 2).reshape(128, 4 * QT))
    ident = np.eye(128, dtype=np.float32).astype(bf16)

    in_maps = []
    for c in range(N_CORES):
        hA = c
        hB = 8 + c // 2
        par = c & 1
        xB = np.ascontiguousarray(
            chunks[par::2].reshape(TB, D_MODEL).T).astype(bf16)
        w = np.zeros((3, D_MODEL, 128), np.float32)
        b = np.zeros((128, 3), np.float32)
        wo2 = np.zeros((128, D_MODEL), np.float32)
        for s, (W, B) in enumerate(((wq, bq), (wk, bk), (wv, bv))):
            w[s, :, 0:64] = W[hA]
            b[0:64, s] = B[hA]
            w[s, :, 64:128] = W[hB]
            b[64:128, s] = B[hB]
        wo2[0:64, :] = wo[hA * 64:(hA + 1) * 64, :]
        # pack w into the device SBUF layout [p, (s*CCH+j)*128 + c]
        w_packed = np.ascontiguousarray(
            w.reshape(3, CCH, 128, 128).transpose(2, 0, 1, 3)
            .reshape(128, 3 * CCH * 128))
        mB_packed = np.ascontiguousarray(
            masks[[par, 2 + par]].transpose(1, 0, 2).reshape(128, 2 * QT))
        in_maps.append({
            "xT": xT,
            "xTB": xB,
            "wqkv": w_packed.astype(bf16),
            "bqkv": b.astype(np.float32),
            "wo2": wo2.astype(bf16),
            "masks": masks_packed,
            "masksB": mB_packed,
            "ident": ident,
        })
    return in_maps


def kernel(_trace=False, _tmpdir=None, **inputs):
    x = np.asarray(inputs["x"], np.float32)
    wo = np.asarray(inputs["wo"], np.float32)
    args = (x,
            np.asarray(inputs["wq"], np.float32), np.asarray(inputs["bq"], np.float32),
            np.asarray(inputs["wk"], np.float32), np.asarray(inputs["bk"], np.float32),
            np.asarray(inputs["wv"], np.float32), np.asarray(inputs["bv"], np.float32),
            wo)
    bo = np.asarray(inputs["bo"], np.float32)

    if "nc" not in _PROGRAM_CACHE:
        _PROGRAM_CACHE["nc"] = build_program()
    nc = _PROGRAM_CACHE["nc"]

    in_maps = _host_inputs(*args)
    res = run_bass_kernel_spmd(
        nc, in_maps, list(range(N_CORES)), trace=_trace, tmpdir=_tmpdir,
    )
    acc = np.zeros((D_MODEL, T), np.float32)
    for c in range(N_CORES):
        acc += res.results[c]["outT"].astype(np.float32)
    out = acc.T                                     # [T, D]
    # join the key-split heads: (av0+av1)/(d0+d1), then out-project
    for m in range(4):
        b0 = np.asarray(res.results[2 * m]["outB"], np.float32)
        b1 = np.asarray(res.results[2 * m + 1]["outB"], np.float32)
        av = b0[0:64] + b1[0:64]                    # [64, T]
        den = b0[64] + b1[64]                       # [T]
        ho = av / den[None, :]
        out += ho.T @ wo[(8 + m) * 64:(9 + m) * 64, :]
    out = out + bo[None, :]
    if _trace:
        return out[None].astype(np.float32), res
    return out[None].astype(np.float32)


# revision 51
# speedup vs baseline: 1.0232x; 1.0232x over previous
"""Multi-head causal attention (B=1, T=4096, D=768, H=12) on 8 trn2 cores.

v3 sharding (balanced, no dummy slots): every core runs the identical
SPMD program with two head-slots:
  slot A (QKV partition rows 0:64) = one of heads 0..7, full attention,
    normalized + out-projected on device -> partial outT (bf16).
  slot B (rows 64:128) = one of heads 8..11, KEY-SPLIT in half across a
    core pair: core parity p handles natural key chunks {2j+p}.  The
    host packs xT_B = x restricted to those 2048 tokens, so on device
    slot B sees a dense 2048-key sequence; for query tile qi it needs
    exactly 2(qi+1) key chunks on every core (shape-uniform).  Slot B's
    unnormalized AV + denominator go back raw (outB); the host joins the
    two halves (av0+av1)/(d0+d1) and applies that head's out-projection
    (a tiny [4096,64]@[64,768] BLAS call per split head).

Per-core causal work: 1 + 0.5 heads = 12/8 of a head -- balanced.

Other structure (from v2): batched DVE reciprocals, gpsimd stride-0
broadcast of 1/denom, av evacuated with one [128,1024] copy into an f32
staging buffer, bf16 outT.
"""

import math
import numpy as np
import ml_dtypes
from contextlib import ExitStack

import concourse.bass as bass
import concourse.bacc as bacc
import concourse.mybir as mybir
import concourse.tile as tile
from concourse.bass_utils import run_bass_kernel_spmd

BF16 = mybir.dt.bfloat16
F32 = mybir.dt.float32
AF = mybir.ActivationFunctionType

T = 4096
TB = T // 2               # slot-B dense key space
D_MODEL = 768
HEAD_DIM = 64
N_HEADS = 12
N_CORES = 8
QT = 512                  # query tile width (one full PSUM bank per chunk)
KC = 128                  # key chunk (psum partition dim)
GRP = 3                   # score chunk-jobs per exp group -> ACT free dim 1536
NQT = T // QT             # 8 query tiles
CCH = D_MODEL // 128      # 6 contraction chunks
TOKT = 512                # token tile for projections
NTOKT = T // TOKT
NTOKTB = TB // TOKT
VST = 130                 # V storage stride per 128-token chunk

_PROGRAM_CACHE = {}


def build_program():
    nc = bacc.Bacc(None)

    # weights/masks arrive host-packed in device layout: one DMA each
    # (every dma_start costs ~650ns of ISSUE time on its queue; 33 small
    # startup DMAs serialized to ~21us of PE-idle before the first matmul)
    xT_d = nc.declare_dram_parameter("xT", [D_MODEL, T], BF16, isOutput=False)
    xTB_d = nc.declare_dram_parameter("xTB", [D_MODEL, TB], BF16, isOutput=False)
    w_d = nc.declare_dram_parameter("wqkv", [128, 3 * CCH * 128], BF16, isOutput=False)
    b_d = nc.declare_dram_parameter("bqkv", [128, 3], F32, isOutput=False)
    wo_d = nc.declare_dram_parameter("wo2", [128, D_MODEL], BF16, isOutput=False)
    mk_d = nc.declare_dram_parameter("masks", [128, 4 * QT], BF16, isOutput=False)
    mkB_d = nc.declare_dram_parameter("masksB", [128, 2 * QT], BF16, isOutput=False)
    id_d = nc.declare_dram_parameter("ident", [128, 128], BF16, isOutput=False)
    outT_d = nc.declare_dram_parameter("outT", [D_MODEL, T], BF16, isOutput=True)
    outB_d = nc.declare_dram_parameter("outB", [65, T], F32, isOutput=True)

    with tile.TileContext(nc) as tc, ExitStack() as ctx:
        consts = ctx.enter_context(tc.tile_pool(name="consts", bufs=1))
        big = ctx.enter_context(tc.tile_pool(name="big", bufs=1))
        ptp = ctx.enter_context(tc.tile_pool(name="ptp", bufs=3))
        osb = ctx.enter_context(tc.tile_pool(name="osb", bufs=3))
        rbp = ctx.enter_context(tc.tile_pool(name="rbp", bufs=8))
        # PSUM: scores/proj/outproj pool 3 banks x2, av 2 banks x1 = 8
        sp = ctx.enter_context(tc.tile_pool(name="sp", bufs=2, space="PSUM"))
        avp = ctx.enter_context(tc.tile_pool(name="avp", bufs=1, space="PSUM"))
        dramp = ctx.enter_context(tc.tile_pool(name="dramp", bufs=1, space="DRAM"))

        # ---- constants / inputs to SBUF ----
        # issue order: weights first (first matmul needs them), then xT on
        # sync; xTB via the idle gpsimd queue; late-needed consts via scalar
        w_sb = consts.tile([128, 3 * CCH * 128], BF16, tag="w")
        nc.sync.dma_start(out=w_sb[:], in_=w_d[:, :])
        b_sb = consts.tile([128, 3], F32, tag="b")
        nc.sync.dma_start(out=b_sb[:], in_=b_d[:, :])
        # xT in halves: projections for token tiles 0-3 start once the first
        # 3.15MB lands instead of waiting for all of xT; xTB queues behind
        # xT on the same engine so it cannot steal HBM from the critical path
        HT = T // 2
        xT_sb = []
        for j in range(CCH):
            t = big.tile([128, T], BF16, tag=f"xT{j}")
            nc.sync.dma_start(
                out=t[:, 0:HT], in_=xT_d[j * 128:(j + 1) * 128, 0:HT])
            xT_sb.append(t)
        for j in range(CCH):
            nc.sync.dma_start(
                out=xT_sb[j][:, HT:T], in_=xT_d[j * 128:(j + 1) * 128, HT:T])
        xTB_sb = []
        for j in range(CCH):
            t = big.tile([128, TB], BF16, tag=f"xTB{j}")
            nc.sync.dma_start(out=t[:], in_=xTB_d[j * 128:(j + 1) * 128, :])
            xTB_sb.append(t)
        wo_sb = consts.tile([128, D_MODEL], BF16, tag="wo")
        nc.scalar.dma_start(out=wo_sb[:], in_=wo_d[:, :])
        mask_sb = consts.tile([128, 4 * QT], BF16, tag="mask")
        nc.scalar.dma_start(out=mask_sb[:], in_=mk_d[:, :])
        maskB_sb = consts.tile([128, 2 * QT], BF16, tag="maskB")
        nc.scalar.dma_start(out=maskB_sb[:], in_=mkB_d[:, :])
        id_sb = consts.tile([128, 128], BF16, tag="id")
        nc.scalar.dma_start(out=id_sb[:], in_=id_d[:, :])

        # ---- projections ----
        # Q: both slots read xT (slot B attends from all 4096 queries).
        # K/V: slot A from xT (4096 keys), slot B from xTB (2048 dense keys).
        qkv_sb = []
        for s in range(3):
            t = big.tile([128, T], BF16, tag=f"qkv{s}")
            qkv_sb.append(t)
        # slot B's K/V region beyond TB is never projected; zero V there so
        # the shared PE transposes read defined data
        nc.vector.memset(qkv_sb[2][64:128, TB:T], 0.0)

        def mm_split(out, lhsT, rhs, first, last, col=0):
            nc.tensor.matmul(
                out, lhsT, rhs, start=first, stop=last,
                tile_position=(0, col),
            )

        for tt in range(NTOKT):
            pp = sp.tile([128, TOKT], F32, tag="sc")
            for j in range(CCH):
                base = j * 128
                mm_split(pp[:, :], w_sb[:, base:base + 128],
                         xT_sb[j][:, tt * TOKT:(tt + 1) * TOKT],
                         j == 0, j == CCH - 1)
            nc.vector.tensor_scalar_add(
                qkv_sb[0][:, tt * TOKT:(tt + 1) * TOKT],
                pp[:, :], b_sb[:, 0:1],
            )
        for s in (1, 2):
            for tt in range(NTOKT):
                if tt < NTOKTB:
                    pp = sp.tile([128, 2 * TOKT], F32, tag="sc")
                    for j in range(CCH):
                        base = (s * CCH + j) * 128
                        mm_split(pp[0:64, 0:TOKT], w_sb[:, base:base + 64],
                                 xT_sb[j][:, tt * TOKT:(tt + 1) * TOKT],
                                 j == 0, j == CCH - 1)
                        mm_split(pp[64:128, TOKT:2 * TOKT],
                                 w_sb[:, base + 64:base + 128],
                                 xTB_sb[j][:, tt * TOKT:(tt + 1) * TOKT],
                                 j == 0, j == CCH - 1, col=64)
                    nc.vector.tensor_scalar_add(
                        qkv_sb[s][0:64, tt * TOKT:(tt + 1) * TOKT],
                        pp[0:64, 0:TOKT], b_sb[0:64, s:s + 1],
                    )
                    nc.vector.tensor_scalar_add(
                        qkv_sb[s][64:128, tt * TOKT:(tt + 1) * TOKT],
                        pp[64:128, TOKT:2 * TOKT], b_sb[64:128, s:s + 1],
                    )
                else:
                    pp = sp.tile([128, TOKT], F32, tag="sc")
                    for j in range(CCH):
                        base = (s * CCH + j) * 128
                        mm_split(pp[0:64, :], w_sb[:, base:base + 64],
                                 xT_sb[j][:, tt * TOKT:(tt + 1) * TOKT],
                                 j == 0, j == CCH - 1)
                    nc.vector.tensor_scalar_add(
                        qkv_sb[s][0:64, tt * TOKT:(tt + 1) * TOKT],
                        pp[0:64, :], b_sb[0:64, s:s + 1],
                    )
        QT_sb, KT_sb, VT_sb = qkv_sb

        # ---- V2 per 128-token key tile, stride 208 cols:
        #   [0:64]=V_A  [64:65]=1  [65:129]=V_B  [129:130]=1
        # lhsT A = cols 0:65  -> psum rows 0:64 AV_A, row 64 denom_A
        # lhsT B = cols 65:130 -> psum rows 0:64 AV_B, row 64 denom_B
        # (both lhsTs 65-wide: LDWEIGHTS cost is per-column, 54ns not 107)
        V_sb = big.tile([128, (T // 128) * VST], BF16, tag="V")
        nc.vector.memset(V_sb[:], 0.0)
        v3 = V_sb[:].rearrange("p (t c) -> p t c", c=VST)
        nc.vector.memset(v3[:, :, 64:65], 1.0)
        nc.vector.memset(v3[:, :, 129:130], 1.0)
        for tt4 in range(T // 128):
            tp = sp.tile([128, 128], BF16, tag="sc")
            nc.tensor.transpose(tp[:], VT_sb[:, tt4 * 128:(tt4 + 1) * 128], id_sb[:])
            nc.vector.tensor_copy(V_sb[:, tt4 * VST:tt4 * VST + 64], tp[:, 0:64])
            nc.vector.tensor_copy(
                V_sb[:, tt4 * VST + 65:tt4 * VST + 129], tp[:, 64:128])

        # ---- attention: scores -> exp -> mask -> AV, denoms staged ----
        # denominator staging: engine ops need base partition 0/32/64/96,
        # so qtiles 0-3 stage at partitions 0:4 and qtiles 4-7 at 32:36
        avs_all = big.tile([128, NQT * 2 * QT], F32, tag="avs")
        den_sb = big.tile([36, QT], F32, tag="den")
        r_sb = big.tile([36, QT], F32, tag="r")
        ho_all = big.tile([64, T], BF16, tag="ho")
        rd = dramp.tile([36, QT], F32, tag="rd")

        def den_row(qi):
            return (0 if qi < 4 else 32) + (qi % 4)

        for qi in range(NQT):
            qs = qi * QT
            av = avp.tile([128, 2 * QT], F32, tag="av")
            nsteps = 4 * (qi + 1)
            nstepsB = 2 * (qi + 1)
            jobs = []
            for i in range(nsteps):
                jobs.append((i, 0))
                if i < nstepsB:
                    jobs.append((i, 1))
            for g in range(0, len(jobs), GRP):
                grp = jobs[g:g + GRP]
                width = len(grp) * QT
                sc = sp.tile([128, GRP * QT], F32, tag="sc")
                for ji, (kc, h) in enumerate(grp):
                    nc.tensor.matmul(
                        sc[:, ji * QT:(ji + 1) * QT],
                        KT_sb[64 * h:64 * h + 64, kc * KC:(kc + 1) * KC],
                        QT_sb[64 * h:64 * h + 64, qs:qs + QT],
                        start=True, stop=True, tile_position=(64 * h, 0),
                    )
                pt = ptp.tile([128, GRP * QT], BF16, tag="pt")
                nc.scalar.activation(
                    pt[:, :width], sc[:, :width], AF.Exp,
                    scale=1.0 / math.sqrt(HEAD_DIM),
                )
                for ji, (kc, h) in enumerate(grp):
                    ptj = pt[:, ji * QT:(ji + 1) * QT]
                    if h == 0:
                        if kc >= 4 * qi:  # diagonal straddle
                            pat = kc - 4 * qi
                            m = mask_sb[:, pat * QT:(pat + 1) * QT]
                            nc.vector.tensor_mul(ptj, ptj, m)
                        vbase = kc * VST
                        nc.tensor.matmul(
                            av[0:65, 0:QT], V_sb[:, vbase:vbase + 65], ptj,
                            start=(kc == 0), stop=(kc == nsteps - 1),
                            tile_position=(0, 0),
                        )
                    else:
                        if kc >= 2 * qi:  # diagonal straddle (2 per qtile)
                            pat = kc - 2 * qi
                            m = maskB_sb[:, pat * QT:(pat + 1) * QT]
                            nc.vector.tensor_mul(ptj, ptj, m)
                        vbase = kc * VST
                        nc.tensor.matmul(
                            av[0:65, QT:2 * QT], V_sb[:, vbase + 65:vbase + 130],
                            ptj, start=(kc == 0), stop=(kc == nstepsB - 1),
                            tile_position=(0, 0),
                        )
            # evacuate av bank pair with one big copy; stage A denom row
            avs = avs_all[:, qi * 2 * QT:(qi + 1) * 2 * QT]
            nc.vector.tensor_copy(avs, av[:, :])
            # DVE lanes cannot cross partitions; gather denom rows into the
            # base-aligned staging tile with tiny SBUF->SBUF DMAs
            ra = den_row(qi)
            nc.sync.dma_start(out=den_sb[ra:ra + 1, :], in_=avs[64:65, 0:QT])
            # slot B raw output (unnormalized av rows 0:64 + denom row 64)
            nc.sync.dma_start(
                out=outB_d[0:65, qs:qs + QT], in_=avs[0:65, QT:2 * QT])
            if qi == 3:
                nc.vector.reciprocal(r_sb[0:4, :], den_sb[0:4, :])
                nc.sync.dma_start(out=rd[0:4, :], in_=r_sb[0:4, :])
            if qi == 7:
                nc.vector.reciprocal(r_sb[32:36, :], den_sb[32:36, :])
                nc.sync.dma_start(out=rd[32:36, :], in_=r_sb[32:36, :])

        # ---- phase B: normalize + out-projection (slot A only) ----
        for qi in range(NQT):
            qs = qi * QT
            avs = avs_all[:, qi * 2 * QT:(qi + 1) * 2 * QT]
            rbc = rbp.tile([64, QT], F32, tag="rbc")
            ra = den_row(qi)
            rdA = rd[ra:ra + 1, :]
            nc.gpsimd.dma_start(
                out=rbc[0:64, :],
                in_=bass.AP(tensor=rdA.tensor, offset=rdA.offset,
                            ap=[[0, 64]] + list(rdA.ap[1:])))
            nc.vector.tensor_mul(
                ho_all[0:64, qs:qs + QT], avs[0:64, 0:QT], rbc[0:64, :])
            # 3 dout-chunks per PSUM tile -> one cast per tile
            for dg in range(CCH // 3):
                op = sp.tile([128, 3 * QT], F32, tag="sc")
                for i in range(3):
                    dch = dg * 3 + i
                    nc.tensor.matmul(
                        op[:, i * QT:(i + 1) * QT],
                        wo_sb[0:64, dch * 128:(dch + 1) * 128],
                        ho_all[0:64, qs:qs + QT], start=True, stop=True,
                    )
                ot = osb.tile([128, 3 * QT], BF16, tag="ot")
                nc.vector.tensor_copy(ot[:], op[:, :])
                # single store: dst viewed [128p, 3i, 512q] to match src
                nc.sync.dma_start(
                    out=outT_d[dg * 384:(dg + 1) * 384, qs:qs + QT]
                        .rearrange("(i p) q -> p i q", i=3),
                    in_=ot[:].rearrange("p (i q) -> p i q", i=3),
                )
    nc.finalize()
    return nc


def _host_inputs(x, wq, bq, wk, bk, wv, bv, wo):
    """Per-core input maps. Slot A of core c = head c (full); slot B =
    head 8 + c//2, key-parity c&1 (half keys, host-packed dense)."""
    bf16 = ml_dtypes.bfloat16
    x0 = np.asarray(x[0], np.float32)              # [T, D]
    xT = np.ascontiguousarray(x0.T).astype(bf16)
    chunks = x0.reshape(T // 128, 128, D_MODEL)
    masks = np.zeros((4, 128, QT), np.float32)
    dk = np.arange(128)[:, None]
    dq = np.arange(QT)[None, :]
    for p in range(4):
        masks[p] = (dk + 128 * p <= dq)
    masks = masks.astype(bf16)
    # device layout [128, pat*QT+q] so each mask set loads as one DMA
    masks_packed = np.ascontiguousarray(
        masks.transpose(1, 0, 2).reshape(128, 4 * QT))
    ident = np.eye(128, dtype=np.float32).astype(bf16)

    in_maps = []
    for c in range(N_CORES):
        hA = c
        hB = 8 + c // 2
        par = c & 1
        xB = np.ascontiguousarray(
            chunks[par::2].reshape(TB, D_MODEL).T).astype(bf16)
        w = np.zeros((3, D_MODEL, 128), np.float32)
        b = np.zeros((128, 3), np.float32)
        wo2 = np.zeros((128, D_MODEL), np.float32)
        for s, (W, B) in enumerate(((wq, bq), (wk, bk), (wv, bv))):
            w[s, :, 0:64] = W[hA]
            b[0:64, s] = B[hA]
            w[s, :, 64:128] = W[hB]
            b[64:128, s] = B[hB]
        wo2[0:64, :] = wo[hA * 64:(hA + 1) * 64, :]
        # pack w into the device SBUF layout [p, (s*CCH+j)*128 + c]
        w_packed = np.ascontiguousarray(
            w.reshape(3, CCH, 128, 128).transpose(2, 0, 1, 3)
            .reshape(128, 3 * CCH * 128))
        mB_packed = np.ascontiguousarray(
            masks[[par, 2 + par]].transpose(1, 0, 2).reshape(128, 2 * QT))
        in_maps.append({
            "xT": xT,
            "xTB": xB,
            "wqkv": w_packed.astype(bf16),
            "bqkv": b.astype(np.float32),
            "wo2": wo2.astype(bf16),
            "masks": masks_packed,
            "masksB": mB_packed,
            "ident": ident,
        })
    return in_maps


def kernel(_trace=False, _tmpdir=None, **inputs):
    x = np.asarray(inputs["x"], np.float32)
    wo = np.asarray(inputs["wo"], np.float32)
    args = (x,
            np.asarray(inputs["wq"], np.float32), np.asarray(inputs["bq"], np.float32),
            np.asarray(inputs["wk"], np.float32), np.asarray(inputs["bk"], np.float32),
            np.asarray(inputs["wv"], np.float32), np.asarray(inputs["bv"], np.float32),
            wo)
    bo = np.asarray(inputs["bo"], np.float32)

    if "nc" not in _PROGRAM_CACHE:
        _PROGRAM_CACHE["nc"] = build_program()
    nc = _PROGRAM_CACHE["nc"]

    in_maps = _host_inputs(*args)
    res = run_bass_kernel_spmd(
        nc, in_maps, list(range(N_CORES)), trace=_trace, tmpdir=_tmpdir,
    )
    acc = np.zeros((D_MODEL, T), np.float32)
    for c in range(N_CORES):
        acc += res.results[c]["outT"].astype(np.float32)
    out = acc.T                                     # [T, D]
    # join the key-split heads: (av0+av1)/(d0+d1), then out-project
    for m in range(4):
        b0 = np.asarray(res.results[2 * m]["outB"], np.float32)
        b1 = np.asarray(res.results[2 * m + 1]["outB"], np.float32)
        av = b0[0:64] + b1[0:64]                    # [64, T]
        den = b0[64] + b1[64]                       # [T]
        ho = av / den[None, :]
        out += ho.T @ wo[(8 + m) * 64:(9 + m) * 64, :]
    out = out + bo[None, :]
    if _trace:
        return out[None].astype(np.float32), res
    return out[None].astype(np.float32)
